# revision 32
# baseline (speedup 1.0000x reference)
"""Trainium2 Bass kernel for an FFM (field-aware factorization machine) layer.

Reference computation (B=16384, P=512, F=16, K=8):
    A[i,j,:] = v[i, f2f[j], :]
    S[i,j]   = sum_k A[i,j,k] * A[j,i,k]          (symmetric)
    rp[b]    = sum_{i<j} x[b,i] * S[i,j] * x[b,j]
    out      = x @ w + rp[:,None] + b

Because S is symmetric, the strictly-upper-triangular quadratic form reduces to
    rp[b] = x[b] @ M @ x[b]^T,   M = 0.5 * (S - diag(S))
so with y' = x @ M + 1*w^T (a plain [512,512] matmul):
    out[b] = sum_j x[b,j] * (y'[b,j]) + bias

Host side folds (v, f2f) -> M and pre-transposes/casts x to fp16 x^T tiles in
the exact (contiguous) layout the device DMAs want; the device does only the
dominant work, data-parallel over batch across 8 NeuronCores:

    per core (batch shard of 2048 rows, transposed orientation):
      y'^T[j,b] accumulated in PSUM from 4 K=128 fp16 matmuls (M chunks
      stationary, x^T moving); DVE scalar_tensor_tensor computes
      z = (y'^T + w) * x^T; basis-vector matmuls reduce z over partitions,
      with all four batch tiles accumulating into one PSUM bank (row = batch
      tile); one DVE tensor_scalar adds the scalar bias; one DMA stores.

Pipeline notes: matmul groups are split into ic-halves so compute starts
after only 0.5 MB has landed; each tile's reduce matmuls are delayed one
phase so their z inputs (DVE) are always ready; PE warmup matmuls run on a
memset tile so they depend on no DMA and keep the HAM clock-gate open until
real data lands.  All HBM reads are whole contiguous blocks.
"""

import time
from contextlib import ExitStack

import numpy as np

import concourse.bass as bass
import concourse.mybir as mybir
import concourse.tile as tile
from concourse import bacc
from concourse.bass import ds, ts
from concourse.bass_utils import run_bass_kernel_spmd

B, P, F, K = 16384, 512, 16, 8
N_CORES = 8
B_SH = B // N_CORES          # 2048 batch rows per core
BT = 512                     # batch tile (free dim of transposed tiles)
NBT = B_SH // BT             # 4 batch tiles per core
NC128 = P // 128             # 4 chunks of 128 along the feature dim

FP32 = mybir.dt.float32
FP16 = mybir.dt.float16

N_WARM = 30                  # PE warmup matmuls (cover barrier->first-data)

# test.py can read this after calling kernel() (exec_time_ns etc.)
LAST_RESULT = None


def _build_nc(bias: float) -> bass.Bass:
    nc = bacc.Bacc("TRN2", target_bir_lowering=False, debug=False,
                   num_devices=N_CORES)

    # xt_d[bt, h, i2, p, bn] = x[bt*512 + bn, (2*h + i2)*128 + p]; every
    # [bt, h, i2] block is a fully contiguous 128 KB region (fp16)
    xt_d = nc.dram_tensor("xt", [NBT, 2, 2, 128, BT], FP16,
                          kind="ExternalInput")
    # m_d[h, i2, p, jc, q] = M[(2*h + i2)*128 + p, jc*128 + q]
    m_d = nc.dram_tensor("m", [2, 2, 128, NC128, 128], FP16,
                         kind="ExternalInput")
    # w_d[p, jc] = w[jc*128 + p]
    w_d = nc.dram_tensor("w", [128, NC128], FP32, kind="ExternalInput")
    out_d = nc.dram_tensor("out", [B_SH, 1], FP32, kind="ExternalOutput")

    with tile.TileContext(nc) as tc, ExitStack() as ctx:
        const = ctx.enter_context(tc.tile_pool(name="const", bufs=1))
        xtp = ctx.enter_context(tc.tile_pool(name="xt", bufs=NBT))
        zp = ctx.enter_context(tc.tile_pool(name="z", bufs=9))
        orp = ctx.enter_context(tc.tile_pool(name="orow", bufs=1))
        pyp = ctx.enter_context(tc.tile_pool(name="py", bufs=7, space="PSUM"))
        prp = ctx.enter_context(tc.tile_pool(name="pr", bufs=1, space="PSUM"))

        # ---- DMA-independent PE warmup: nonzero memset tile (the HAM
        # activity monitor needs real data toggling), then dense matmuls.
        # wps borrows a py-pool slot and releases it when warmup ends.
        warm = const.tile([128, 128], FP16)
        nc.vector.memset(warm[:], 1.0)
        wps = pyp.tile([128, BT], FP32, tag="py", name="wps")
        for _ in range(N_WARM):
            nc.tensor.matmul(wps[:, :128], lhsT=warm[:], rhs=warm[:],
                             start=True, stop=True)

        # eb[:, 3-bt : 7-bt] is the 128x4 stationary whose bt-th column is
        # ones -> the reduce matmul lands tile bt's result in PSUM row bt.
        eb = const.tile([128, 8], FP16)
        nc.vector.memset(eb[:], 0.0)
        nc.vector.memset(eb[:, 3:4], 1.0)

        # The Sync HWDGE queue drains FIFO in issue order, so issue the
        # first-needed contiguous blocks (M ic-half A, x0 ic-half A) first.
        # The w load and the out store ride the Scalar HWDGE ring.
        mt = const.tile([128, NC128, NC128, 128], FP16)   # [p, ic, jc, q]
        wt = const.tile([128, NC128], FP32)

        xts = []
        for bt in range(NBT):
            xt = xtp.tile([128, NC128, BT], FP16, tag="xt", name=f"xt_{bt}")
            xts.append(xt)
        m_src = m_d.ap().rearrange("h i2 p jc q -> p h i2 jc q")
        x_src = xt_d.ap().rearrange("bt h i2 p bn -> p bt h i2 bn")
        # finest chunks first: the opening matmuls are gated on only 256 KB.
        # The tiny w load doubles as a ring primer: it absorbs the ~1.5 us
        # cold-start of the DMA path before the critical M/x chunks flow.
        nc.sync.dma_start(wt[:], w_d.ap())
        nc.sync.dma_start(mt[:, 0:1], m_src[:, 0, 0:1])
        nc.sync.dma_start(xts[0][:, 0:1], x_src[:, 0, 0, 0:1])
        nc.sync.dma_start(mt[:, 1:2], m_src[:, 0, 1:2])
        nc.sync.dma_start(xts[0][:, 1:2], x_src[:, 0, 0, 1:2])
        nc.sync.dma_start(mt[:, 2:4], m_src[:, 1])
        nc.sync.dma_start(xts[0][:, 2:4], x_src[:, 0, 1])
        nc.sync.dma_start(xts[1][:, 0:2], x_src[:, 1, 0])
        nc.sync.dma_start(xts[1][:, 2:4], x_src[:, 1, 1])
        for bt in (2, 3):
            dst = xts[bt][:].rearrange("p (h i2) bn -> p h i2 bn", h=2)
            nc.sync.dma_start(dst, x_src[:, bt])

        orow = orp.tile([NBT, BT], FP32)
        pr = prp.tile([NBT, BT], FP32)

        # ---- main pipeline ----
        # Per tile: two ic-half phases of y-matmuls (jc-inner), STT drains as
        # soon as each py completes, and the tile's reduce matmuls are
        # emitted one phase later so z is always ready when PE gets there.
        pending_reduce = []            # (bt, zs) awaiting reduce emission
        n_red = 0

        def emit_reduce():
            nonlocal n_red
            if not pending_reduce:
                return
            bt, zs = pending_reduce.pop(0)
            for jc, z in enumerate(zs):
                nc.tensor.matmul(pr[:], lhsT=eb[:, ds(3 - bt, 4)], rhs=z[:],
                                 start=(n_red == 0),
                                 stop=(n_red == NBT * NC128 - 1))
                n_red += 1

        for bt in range(NBT):
            xt = xts[bt]
            pys = [pyp.tile([128, BT], FP32, tag="py", name=f"py_{bt}_{j}")
                   for j in range(NC128)]
            zs = []
            for ic in range(NC128):
                for jc in range(NC128):
                    nc.tensor.matmul(pys[jc][:], lhsT=mt[:, ic, jc, :],
                                     rhs=xt[:, ic, :],
                                     start=(ic == 0),
                                     stop=(ic == NC128 - 1))
                    if ic == NC128 - 1:
                        # drain py as soon as it completes
                        z = zp.tile([128, BT], FP16, tag="z",
                                    name=f"z_{bt}_{jc}")
                        nc.vector.scalar_tensor_tensor(
                            out=z[:], in0=pys[jc][:],
                            scalar=wt[:, jc:jc + 1], in1=xt[:, jc, :],
                            op0=mybir.AluOpType.add,
                            op1=mybir.AluOpType.mult)
                        zs.append(z)
            emit_reduce()
            pending_reduce.append((bt, zs))
        while pending_reduce:
            emit_reduce()

        # bias add + PSUM->SBUF move on ACT (same engine then issues the
        # store -- no extra cross-engine hop); single store for all tiles
        nc.scalar.activation(orow[:], pr[:],
                             mybir.ActivationFunctionType.Copy,
                             bias=float(bias), scale=1.0)
        nc.scalar.dma_start(
            out_d.ap().rearrange("(t b) one -> t (one b)", t=NBT), orow[:])

    nc.compile()
    return nc


def kernel(x: np.ndarray, w: np.ndarray, v: np.ndarray, b: np.ndarray,
           f2f: np.ndarray) -> np.ndarray:
    global LAST_RESULT
    x = np.asarray(x, dtype=np.float32)
    w = np.asarray(w, dtype=np.float32)
    v = np.asarray(v, dtype=np.float32)
    b = np.asarray(b, dtype=np.float32)
    f2f = np.asarray(f2f, dtype=np.int32)

    # ---- host: fold (v, f2f) into the interaction matrix M ----
    A = v[:, f2f, :]                                # [P, P, K]
    S = np.einsum('ijk,jik->ij', A, A)              # [P, P], symmetric
    M = 0.5 * (S - np.diag(np.diag(S)))             # strict-triu quadratic form

    # m_host[h, i2, p, jc, q] = M[(2h+i2)*128 + p, jc*128 + q]
    m_host = np.ascontiguousarray(
        M.reshape(2, 2, 128, NC128, 128)            # [h, i2, p, jc, q]
        .astype(np.float16))
    w_host = np.ascontiguousarray(
        w[:, 0].reshape(NC128, 128).T.astype(np.float32))  # [128, NC128]
    bias = float(b[0])

    # xt_host[c, bt, h, p, i2, bn] = x[c*2048 + bt*512 + bn, (2h+i2)*128 + p]
    xt_all = np.ascontiguousarray(
        x.astype(np.float16)
        .reshape(N_CORES, NBT, BT, 2, 2, 128)       # [c, bt, bn, h, i2, p]
        .transpose(0, 1, 3, 4, 5, 2))               # [c, bt, h, i2, p, bn]

    nc = _build_nc(bias)

    in_maps = []
    for c in range(N_CORES):
        in_maps.append({
            "xt": xt_all[c],
            "m": m_host,
            "w": w_host,
        })

    res = None
    last_exc = None
    out = None
    for attempt in range(4):
        try:
            res = run_bass_kernel_spmd(nc, in_maps,
                                       core_ids=list(range(N_CORES)))
            # materialize inside the try: async device errors surface here
            out = np.concatenate([np.asarray(r["out"]) for r in res.results],
                                 axis=0)
            break
        except Exception as exc:           # transient NRT/device hiccups
            last_exc = exc
            try:
                import jax
                jax.clear_caches()
                jax.extend.backend.clear_backends()
            except Exception:
                pass
            time.sleep(5.0)
    if out is None:
        raise last_exc
    LAST_RESULT = res

    return out.astype(np.float32)


if __name__ == "__main__":
    rng = np.random.default_rng(0)
    xs = rng.standard_normal((B, P), dtype=np.float32)
    ws = (rng.standard_normal((P, 1)) * 0.05).astype(np.float32)
    vs = (rng.standard_normal((P, F, K)) * 0.05).astype(np.float32)
    bs = rng.standard_normal((1,)).astype(np.float32)
    fs = rng.integers(0, F, size=(P,)).astype(np.int32)
    o = kernel(x=xs, w=ws, v=vs, b=bs, f2f=fs)
    print("out", o.shape, o.dtype, o[:4, 0])


# revision 33
# speedup vs baseline: 1.0639x; 1.0639x over previous
"""Trainium2 Bass kernel for an FFM (field-aware factorization machine) layer.

Reference computation (B=16384, P=512, F=16, K=8):
    A[i,j,:] = v[i, f2f[j], :]
    S[i,j]   = sum_k A[i,j,k] * A[j,i,k]          (symmetric)
    rp[b]    = sum_{i<j} x[b,i] * S[i,j] * x[b,j]
    out      = x @ w + rp[:,None] + b

Because S is symmetric, the strictly-upper-triangular quadratic form reduces to
    rp[b] = x[b] @ M @ x[b]^T,   M = 0.5 * (S - diag(S))
so with y' = x @ M + 1*w^T (a plain [512,512] matmul):
    out[b] = sum_j x[b,j] * (y'[b,j]) + bias

Host side folds (v, f2f) -> M and pre-transposes/casts x to fp16 x^T tiles in
the exact (contiguous) layout the device DMAs want; the device does only the
dominant work, data-parallel over batch across 8 NeuronCores:

    per core (batch shard of 2048 rows, transposed orientation):
      y'^T[j,b] accumulated in PSUM from 4 K=128 fp16 matmuls (M chunks
      stationary, x^T moving); DVE scalar_tensor_tensor computes
      z = (y'^T + w) * x^T; basis-vector matmuls reduce z over partitions,
      with all four batch tiles accumulating into one PSUM bank (row = batch
      tile); one DVE tensor_scalar adds the scalar bias; one DMA stores.

Pipeline notes: matmul groups are split into ic-halves so compute starts
after only 0.5 MB has landed; each tile's reduce matmuls are delayed one
phase so their z inputs (DVE) are always ready; PE warmup matmuls run on a
memset tile so they depend on no DMA and keep the HAM clock-gate open until
real data lands.  All HBM reads are whole contiguous blocks.
"""

import time
from contextlib import ExitStack

import numpy as np

import concourse.bass as bass
import concourse.mybir as mybir
import concourse.tile as tile
from concourse import bacc
from concourse.bass import ds, ts
from concourse.bass_utils import run_bass_kernel_spmd

B, P, F, K = 16384, 512, 16, 8
N_CORES = 8
B_SH = B // N_CORES          # 2048 batch rows per core
BT = 512                     # batch tile (free dim of transposed tiles)
NBT = B_SH // BT             # 4 batch tiles per core
NC128 = P // 128             # 4 chunks of 128 along the feature dim

FP32 = mybir.dt.float32
FP16 = mybir.dt.float16

N_WARM = 30                  # PE warmup matmuls (cover barrier->first-data)

# test.py can read this after calling kernel() (exec_time_ns etc.)
LAST_RESULT = None


def _build_nc(bias: float) -> bass.Bass:
    nc = bacc.Bacc("TRN2", target_bir_lowering=False, debug=False,
                   num_devices=N_CORES)

    # xt_d[bt, h, p, i2, bn] = x[bt*512 + bn, (2*h + i2)*128 + p]; each
    # [bt, h] block is a contiguous 256 KB region (fp16, host-prepared)
    xt_d = nc.dram_tensor("xt", [NBT, 2, 128, 2, BT], FP16,
                          kind="ExternalInput")
    # m_d[h, p, i2, jc, q] = M[(2*h + i2)*128 + p, jc*128 + q]
    m_d = nc.dram_tensor("m", [2, 128, 2, NC128, 128], FP16,
                         kind="ExternalInput")
    # w_d[p, jc] = w[jc*128 + p]
    w_d = nc.dram_tensor("w", [128, NC128], FP32, kind="ExternalInput")
    out_d = nc.dram_tensor("out", [B_SH, 1], FP32, kind="ExternalOutput")

    with tile.TileContext(nc) as tc, ExitStack() as ctx:
        const = ctx.enter_context(tc.tile_pool(name="const", bufs=1))
        xtp = ctx.enter_context(tc.tile_pool(name="xt", bufs=NBT))
        zp = ctx.enter_context(tc.tile_pool(name="z", bufs=9))
        orp = ctx.enter_context(tc.tile_pool(name="orow", bufs=1))
        pyp = ctx.enter_context(tc.tile_pool(name="py", bufs=7, space="PSUM"))
        prp = ctx.enter_context(tc.tile_pool(name="pr", bufs=1, space="PSUM"))

        # ---- DMA-independent PE warmup: nonzero memset tile (the HAM
        # activity monitor needs real data toggling), then dense matmuls.
        # wps borrows a py-pool slot and releases it when warmup ends.
        warm = const.tile([128, 128], FP16)
        nc.vector.memset(warm[:], 1.0)
        wps = pyp.tile([128, BT], FP32, tag="py", name="wps")
        for _ in range(N_WARM):
            nc.tensor.matmul(wps[:, :128], lhsT=warm[:], rhs=warm[:],
                             start=True, stop=True)

        # eb[:, 3-bt : 7-bt] is the 128x4 stationary whose bt-th column is
        # ones -> the reduce matmul lands tile bt's result in PSUM row bt.
        eb = const.tile([128, 8], FP16)
        nc.vector.memset(eb[:], 0.0)
        nc.vector.memset(eb[:, 3:4], 1.0)

        # The Sync HWDGE queue drains FIFO in issue order, so issue the
        # first-needed contiguous blocks (M ic-half A, x0 ic-half A) first.
        # The w load and the out store ride the Scalar HWDGE ring.
        mt = const.tile([128, NC128, NC128, 128], FP16)   # [p, ic, jc, q]
        wt = const.tile([128, NC128], FP32)

        # x tiles 2+3 share one buffer loaded by a single DMA: the completion
        # latency (~3 us from last byte to semaphore visibility) is paid once,
        # and by then the stream is far ahead of the PE's consumption.
        x23 = const.tile([128, 2, NC128, BT], FP16)
        xts = []
        for bt in range(2):
            xt = xtp.tile([128, NC128, BT], FP16, tag="xt", name=f"xt_{bt}")
            xts.append(xt)
        xts.append(x23[:, 0])
        xts.append(x23[:, 1])
        m_src = m_d.ap().rearrange("h p i2 jc q -> p h i2 jc q")
        x_src = xt_d.ap().rearrange("bt h p i2 bn -> p bt h i2 bn")
        # finest chunks first: the opening matmuls are gated on only 256 KB.
        # The tiny w load doubles as a ring primer: it absorbs the ~1.5 us
        # cold-start of the DMA path before the critical M/x chunks flow.
        nc.sync.dma_start(wt[:], w_d.ap())
        nc.sync.dma_start(mt[:, 0:1], m_src[:, 0, 0:1])
        nc.sync.dma_start(xts[0][:, 0:1], x_src[:, 0, 0, 0:1])
        nc.sync.dma_start(mt[:, 1:2], m_src[:, 0, 1:2])
        nc.sync.dma_start(xts[0][:, 1:2], x_src[:, 0, 0, 1:2])
        nc.sync.dma_start(mt[:, 2:4], m_src[:, 1])
        nc.sync.dma_start(xts[0][:, 2:4], x_src[:, 0, 1])
        nc.sync.dma_start(xts[1][:, 0:2], x_src[:, 1, 0])
        nc.sync.dma_start(xts[1][:, 2:4], x_src[:, 1, 1])
        dst23 = x23[:].rearrange("p b2 (h i2) bn -> p b2 h i2 bn", h=2)
        nc.sync.dma_start(dst23, x_src[:, 2:4])

        orow = orp.tile([NBT, BT], FP32)
        pr = prp.tile([NBT, BT], FP32)

        # ---- main pipeline ----
        # Per tile: two ic-half phases of y-matmuls (jc-inner), STT drains as
        # soon as each py completes, and the tile's reduce matmuls are
        # emitted one phase later so z is always ready when PE gets there.
        pending_reduce = []            # (bt, zs) awaiting reduce emission
        n_red = 0

        def emit_reduce():
            nonlocal n_red
            if not pending_reduce:
                return
            bt, zs = pending_reduce.pop(0)
            for jc, z in enumerate(zs):
                nc.tensor.matmul(pr[:], lhsT=eb[:, ds(3 - bt, 4)], rhs=z[:],
                                 start=(n_red == 0),
                                 stop=(n_red == NBT * NC128 - 1))
                n_red += 1

        for bt in range(NBT):
            xt = xts[bt]
            pys = [pyp.tile([128, BT], FP32, tag="py", name=f"py_{bt}_{j}")
                   for j in range(NC128)]
            zs = []
            for ic in range(NC128):
                for jc in range(NC128):
                    nc.tensor.matmul(pys[jc][:], lhsT=mt[:, ic, jc, :],
                                     rhs=xt[:, ic, :],
                                     start=(ic == 0),
                                     stop=(ic == NC128 - 1))
                    if ic == NC128 - 1:
                        # drain py as soon as it completes
                        z = zp.tile([128, BT], FP16, tag="z",
                                    name=f"z_{bt}_{jc}")
                        nc.vector.scalar_tensor_tensor(
                            out=z[:], in0=pys[jc][:],
                            scalar=wt[:, jc:jc + 1], in1=xt[:, jc, :],
                            op0=mybir.AluOpType.add,
                            op1=mybir.AluOpType.mult)
                        zs.append(z)
            emit_reduce()
            pending_reduce.append((bt, zs))
        while pending_reduce:
            emit_reduce()

        # bias add + PSUM->SBUF move on ACT (same engine then issues the
        # store -- no extra cross-engine hop); single store for all tiles
        nc.scalar.activation(orow[:], pr[:],
                             mybir.ActivationFunctionType.Copy,
                             bias=float(bias), scale=1.0)
        nc.scalar.dma_start(
            out_d.ap().rearrange("(t b) one -> t (one b)", t=NBT), orow[:])

    nc.compile()
    return nc


def kernel(x: np.ndarray, w: np.ndarray, v: np.ndarray, b: np.ndarray,
           f2f: np.ndarray) -> np.ndarray:
    global LAST_RESULT
    x = np.asarray(x, dtype=np.float32)
    w = np.asarray(w, dtype=np.float32)
    v = np.asarray(v, dtype=np.float32)
    b = np.asarray(b, dtype=np.float32)
    f2f = np.asarray(f2f, dtype=np.int32)

    # ---- host: fold (v, f2f) into the interaction matrix M ----
    A = v[:, f2f, :]                                # [P, P, K]
    S = np.einsum('ijk,jik->ij', A, A)              # [P, P], symmetric
    M = 0.5 * (S - np.diag(np.diag(S)))             # strict-triu quadratic form

    # m_host[h, p, i2, jc, q] = M[(2h+i2)*128 + p, jc*128 + q]
    m_host = np.ascontiguousarray(
        M.reshape(2, 2, 128, NC128, 128)            # [h, i2, p, jc, q]
        .transpose(0, 2, 1, 3, 4)
        .astype(np.float16))
    w_host = np.ascontiguousarray(
        w[:, 0].reshape(NC128, 128).T.astype(np.float32))  # [128, NC128]
    bias = float(b[0])

    # xt_host[c, bt, h, p, i2, bn] = x[c*2048 + bt*512 + bn, (2h+i2)*128 + p]
    xt_all = np.ascontiguousarray(
        x.astype(np.float16)
        .reshape(N_CORES, NBT, BT, 2, 2, 128)       # [c, bt, bn, h, i2, p]
        .transpose(0, 1, 3, 5, 4, 2))               # [c, bt, h, p, i2, bn]

    nc = _build_nc(bias)

    in_maps = []
    for c in range(N_CORES):
        in_maps.append({
            "xt": xt_all[c],
            "m": m_host,
            "w": w_host,
        })

    res = None
    last_exc = None
    out = None
    for attempt in range(4):
        try:
            res = run_bass_kernel_spmd(nc, in_maps,
                                       core_ids=list(range(N_CORES)))
            # materialize inside the try: async device errors surface here
            out = np.concatenate([np.asarray(r["out"]) for r in res.results],
                                 axis=0)
            break
        except Exception as exc:           # transient NRT/device hiccups
            last_exc = exc
            try:
                import jax
                jax.clear_caches()
                jax.extend.backend.clear_backends()
            except Exception:
                pass
            time.sleep(5.0)
    if out is None:
        raise last_exc
    LAST_RESULT = res

    return out.astype(np.float32)


if __name__ == "__main__":
    rng = np.random.default_rng(0)
    xs = rng.standard_normal((B, P), dtype=np.float32)
    ws = (rng.standard_normal((P, 1)) * 0.05).astype(np.float32)
    vs = (rng.standard_normal((P, F, K)) * 0.05).astype(np.float32)
    bs = rng.standard_normal((1,)).astype(np.float32)
    fs = rng.integers(0, F, size=(P,)).astype(np.int32)
    o = kernel(x=xs, w=ws, v=vs, b=bs, f2f=fs)
    print("out", o.shape, o.dtype, o[:4, 0])
